# revision 11
# baseline (speedup 1.0000x reference)
"""Trainium2 Bass kernel for nn_DHSRNN (dendritic hierarchical spiking RNN).

Strategy: 8-way tensor-parallel over the HID*BRANCH=4096 dendritic feature dim
(512 feats / 128 hidden neurons per core), full batch (128) kept on every core.
Recurrent spikes are exchanged once per timestep with an AllGather of each
core's (128 hid x 128 batch) bf16 spike tile.

v2 layout (vs the earlier weight-stationary version): the gathered spike tile
gt is used as the PE *stationary* operand for the dendritic-drive matmul, with
the dense weights streaming as the moving operand (8 matmuls of free=512
instead of 32 weight-loads of free=128) -- the PE was LDWEIGHTS-bound.  The
dendritic state e lives transposed as (batch x feature); branch sums for the
soma drive are computed on the vector engine and injected into the soma PSUM
with a single transpose-matmul.  Only the 8 wmem matmuls + threshold sit on
the serial path between gather t-1 and gather t.

Math restructuring (exact, same as the validated baseline):
 - biases eliminated via state shifts e = d - b, f = mem - c; spike condition
   is a per-partition threshold f > VTH - c; readout bias re-added on host.
 - soft reset folded: z = f - (VTH/alpha)*spk, f_t = alpha*z_{t-1} + drive.
 - (1-beta) folded into dense weights; the soma drive is
   alpha*z + s(e_{t-1}) + wmem@spk_{t-1} + xm_t, where s is the
   (1-alpha)*beta branch sum of e and xm (x part) is precomputed; xm/alpha is
   folded into z' = z + xm_{t+1}/alpha so no PE identity-matmul is needed.
 - input drive (x @ Wx') for all 200 steps precomputed on-device in phase B;
   streamed back per-step via a transposing DMA into (batch x feature) layout.

Feature order within a core is branch-major (j*128 + h) so branch sums are
contiguous 128-column block adds on the vector engine.
"""
import sys

sys.path.insert(0, "/opt/trn_rl_repo")

import numpy as np
import ml_dtypes

IN_DIM, HID, OUT, BRANCH = 512, 1024, 256, 4
B, T = 128, 200
VTH, WARMUP = 1.0, 10
N_CORES = 8
FPC = HID * BRANCH // N_CORES   # 512 feats per core
HPC = HID // N_CORES            # 128 hid per core
NPIECE = T * B // 512           # 50 phase-B column pieces

bf16 = ml_dtypes.bfloat16

_PROG_CACHE = {}


def _sigmoid(x):
    return (1.0 / (1.0 + np.exp(-np.asarray(x, np.float64)))).astype(np.float32)


def build_program():
    from concourse import bacc, tile, mybir

    nc = bacc.Bacc("TRN2", target_bir_lowering=False, debug=False,
                   num_devices=N_CORES)
    f32 = mybir.dt.float32
    b16 = mybir.dt.bfloat16

    # ---- I/O ----
    whT_in = nc.dram_tensor("whT_in", [HID, FPC], b16, kind="ExternalInput").ap()
    wxT_in = nc.dram_tensor("wxT_in", [IN_DIM, FPC], b16, kind="ExternalInput").ap()
    wmemT_in = nc.dram_tensor("wmemT_in", [HID, HPC], b16, kind="ExternalInput").ap()
    wxmemT_in = nc.dram_tensor("wxmemT_in", [IN_DIM, HPC], b16, kind="ExternalInput").ap()
    wroT_in = nc.dram_tensor("wroT_in", [HPC, OUT], b16, kind="ExternalInput").ap()
    xT_in = nc.dram_tensor("xT_in", [IN_DIM, T * B], b16, kind="ExternalInput").ap()
    einitT_in = nc.dram_tensor("einitT_in", [B, FPC], f32, kind="ExternalInput").ap()
    zinit_in = nc.dram_tensor("zinit_in", [HPC, B], f32, kind="ExternalInput").ap()
    alpha_in = nc.dram_tensor("alpha_in", [HPC, 1], f32, kind="ExternalInput").ap()
    nvoa_in = nc.dram_tensor("nvoa_in", [HPC, 1], f32, kind="ExternalInput").ap()
    thr_in = nc.dram_tensor("thr_in", [HPC, 1], f32, kind="ExternalInput").ap()
    betab_in = nc.dram_tensor("betab_in", [B, FPC], f32, kind="ExternalInput").ap()
    cb_in = nc.dram_tensor("cb_in", [B, FPC], f32, kind="ExternalInput").ap()
    alo_in = nc.dram_tensor("alo_in", [HPC, 2], f32, kind="ExternalInput").ap()
    identb_in = nc.dram_tensor("identb_in", [B, B], b16, kind="ExternalInput").ap()

    acc_out = nc.dram_tensor("acc_out", [HPC, OUT], f32, kind="ExternalOutput").ap()
    junk_out = nc.dram_tensor("junk_out", [HPC, 256], f32, kind="ExternalOutput").ap()

    KT = HID // HPC       # 8 hid k-chunks
    KX = IN_DIM // HPC    # 4 input k-chunks
    MT = FPC // HPC       # 4 feat m-chunks
    Add = mybir.AluOpType.add
    Mult = mybir.AluOpType.mult
    IsGt = mybir.AluOpType.is_gt
    Bypass = mybir.AluOpType.bypass

    with tile.TileContext(nc) as tc:
        with tc.tile_pool(name="consts", bufs=1) as cpool, \
             tc.tile_pool(name="state", bufs=1) as spool, \
             tc.tile_pool(name="dramw", bufs=1, space="DRAM") as dpool:

            # ---- resident constants in SBUF ----
            whT_sb = cpool.tile([HPC, KT * FPC], b16)      # (128, 8*512)
            for k in range(KT):
                nc.sync.dma_start(whT_sb[:, k * FPC:(k + 1) * FPC],
                                  whT_in[k * HPC:(k + 1) * HPC, :])
            wmemT_sb = cpool.tile([HPC, KT * HPC], b16)    # (128, 8*128)
            for k in range(KT):
                nc.sync.dma_start(wmemT_sb[:, k * HPC:(k + 1) * HPC],
                                  wmemT_in[k * HPC:(k + 1) * HPC, :])
            wxT_sb = cpool.tile([HPC, KX * FPC], b16)      # (128, 4*512)
            for k in range(KX):
                nc.sync.dma_start(wxT_sb[:, k * FPC:(k + 1) * FPC],
                                  wxT_in[k * HPC:(k + 1) * HPC, :])
            wxmemT_sb = cpool.tile([HPC, KX * HPC], b16)   # (128, 4*128)
            for k in range(KX):
                nc.sync.dma_start(wxmemT_sb[:, k * HPC:(k + 1) * HPC],
                                  wxmemT_in[k * HPC:(k + 1) * HPC, :])
            wroT_sb = cpool.tile([HPC, OUT], b16)
            nc.sync.dma_start(wroT_sb[:], wroT_in[:])
            identb_sb = cpool.tile([B, B], b16)
            nc.sync.dma_start(identb_sb[:], identb_in[:])
            alpha_sb = cpool.tile([HPC, 1], f32)
            nc.sync.dma_start(alpha_sb[:], alpha_in[:])
            nvoa_sb = cpool.tile([HPC, 1], f32)
            nc.sync.dma_start(nvoa_sb[:], nvoa_in[:])
            thr_sb = cpool.tile([HPC, 1], f32)
            nc.sync.dma_start(thr_sb[:], thr_in[:])
            betab_sb = cpool.tile([B, FPC], f32)
            nc.scalar.dma_start(betab_sb[:], betab_in[:])
            cb_sb = cpool.tile([B, FPC], f32)
            nc.scalar.dma_start(cb_sb[:], cb_in[:])
            alo_sb = cpool.tile([HPC, 2], f32)
            nc.sync.dma_start(alo_sb[:], alo_in[:])

            # ---- persistent state ----
            e_sb = spool.tile([B, FPC], f32)               # (128 batch, 512 feat)
            nc.scalar.dma_start(e_sb[:], einitT_in[:])
            z_sb = spool.tile([HPC, B], f32)               # z' = z + xm/alpha
            nc.sync.dma_start(z_sb[:], zinit_in[:])
            ce_sb = spool.tile([B, FPC], f32)
            t1_sb = spool.tile([B, 2 * HPC], f32)
            g_sb = spool.tile([HPC, OUT], f32)
            nc.vector.memset(g_sb[:], 0.0)
            acc_sb = spool.tile([HPC, OUT], f32)
            nc.vector.memset(acc_sb[:], 0.0)

            # ---- internal DRAM ----
            xdrive_dram = dpool.tile([T, FPC, B], b16)
            xmal_dram = dpool.tile([T, HPC, B], b16)

            # ================= Phase B: x-drive precompute =================
            with tc.tile_pool(name="pbx", bufs=3) as pbx, \
                 tc.tile_pool(name="pbo", bufs=4) as pbo, \
                 tc.tile_pool(name="pbp", bufs=2, space="PSUM") as pbp:
                for p in range(NPIECE):
                    cs = p * 512
                    xtile = pbx.tile([HPC, KX * 512], b16)
                    nc.sync.dma_start(
                        xtile[:].rearrange("p (k c) -> p k c", k=KX),
                        xT_in[:, cs:cs + 512].rearrange("(k p) c -> p k c", k=KX))
                    for m in range(MT):
                        xdp = pbp.tile([HPC, 512], f32, tag="xdp")
                        for k in range(KX):
                            nc.tensor.matmul(
                                xdp[:],
                                lhsT=wxT_sb[:, k * FPC + m * HPC:k * FPC + (m + 1) * HPC],
                                rhs=xtile[:, k * 512:(k + 1) * 512],
                                start=(k == 0), stop=(k == KX - 1))
                        xdo = pbo.tile([HPC, 512], b16, tag="xdo")
                        nc.vector.tensor_copy(xdo[:], xdp[:])
                        nc.scalar.dma_start(
                            xdrive_dram[4 * p:4 * p + 4,
                                        m * HPC:(m + 1) * HPC, :].rearrange(
                                "tl p b -> p tl b"),
                            xdo[:].rearrange("p (tl b) -> p tl b", tl=4))
                    xmp = pbp.tile([HPC, 512], f32, tag="xmp")
                    for k in range(KX):
                        nc.tensor.matmul(
                            xmp[:],
                            lhsT=wxmemT_sb[:, k * HPC:(k + 1) * HPC],
                            rhs=xtile[:, k * 512:(k + 1) * 512],
                            start=(k == 0), stop=(k == KX - 1))
                    xmo = pbo.tile([HPC, 512], b16, tag="xmo")
                    nc.scalar.copy(xmo[:], xmp[:])
                    nc.gpsimd.dma_start(
                        xmal_dram[4 * p:4 * p + 4].rearrange("tl p b -> p tl b"),
                        xmo[:].rearrange("p (tl b) -> p tl b", tl=4))

            # ================= Phase C: recurrent loop =================
            NFILL = 10
            with tc.tile_pool(name="lio", bufs=4) as lio, \
                 tc.tile_pool(name="lgt", bufs=2) as lgt, \
                 tc.tile_pool(name="lfs", bufs=2) as lfs, \
                 tc.tile_pool(name="lss", bufs=2) as lss, \
                 tc.tile_pool(name="ldr", bufs=2, space="DRAM") as ldr, \
                 tc.tile_pool(name="vp", bufs=2, space="PSUM") as vpp, \
                 tc.tile_pool(name="dp", bufs=2, space="PSUM") as dpp, \
                 tc.tile_pool(name="jp", bufs=1, space="PSUM") as jpp, \
                 tc.tile_pool(name="rp", bufs=2, space="PSUM") as rpp:

                junk = jpp.tile([HPC, 256], f32, tag="junk")

                def fetch_inputs(t):
                    xdr = lio.tile([B, FPC], b16, tag="xdr")
                    nc.sync.dma_start(xdr[:], xdrive_dram[t], transpose=True)
                    xmal = lio.tile([HPC, B], b16, tag="xmal")
                    nc.scalar.dma_start(xmal[:], xmal_dram[t])
                    return xdr, xmal

                pre = [fetch_inputs(t) for t in range(2)]

                # initial z' = zinit + xmal_0
                nc.vector.tensor_tensor(z_sb[:], z_sb[:], pre[0][1][:], op=Add)

                # initial s from e_init, into dp for t=0 (no wmem at t=0)
                nc.vector.tensor_tensor(ce_sb[:], e_sb[:], cb_sb[:], op=Mult)
                nc.vector.tensor_tensor(t1_sb[:], ce_sb[:, :2 * HPC],
                                        ce_sb[:, 2 * HPC:], op=Add)
                s_sb = lss.tile([B, HPC], b16, tag="s")
                nc.vector.tensor_tensor(s_sb[:], t1_sb[:, :HPC],
                                        t1_sb[:, HPC:], op=Add)
                dp = dpp.tile([HPC, B], f32, tag="dp")
                nc.tensor.matmul(dp[:], lhsT=s_sb[:], rhs=identb_sb[:],
                                 start=True, stop=True)

                gt_prev = None   # list of 4 tiles, each [HPC, 2*B]
                for t in range(T):
                    xdr, xmal = pre[t]

                    # ---- soma drive: wmem @ gathered spikes (t-1) ----
                    if t > 0:
                        for k in range(KT):
                            nc.tensor.matmul(
                                dp[:],
                                lhsT=wmemT_sb[:, k * HPC:(k + 1) * HPC],
                                rhs=gt_prev[k // 2][:, (k % 2) * B:
                                                    (k % 2 + 1) * B],
                                start=False, stop=(k == KT - 1))

                    # ---- f, spike ----
                    u = lfs.tile([HPC, B], f32, tag="u")
                    nc.vector.scalar_tensor_tensor(
                        u[:], in0=z_sb[:], scalar=alpha_sb[:], in1=dp[:],
                        op0=Mult, op1=Add)
                    spk = lfs.tile([HPC, B], b16, tag="spk")
                    nc.vector.tensor_scalar(spk[:], u[:], thr_sb[:], None,
                                            op0=IsGt)

                    # ---- bounce + all-gather + chunked reload ----
                    if t < T - 1:
                        spkb = ldr.tile([HPC, B], b16, tag="spkb")
                        gout = ldr.tile([HID, B], b16, tag="gout",
                                        addr_space="Shared")
                        nc.scalar.dma_start(spkb[:], spk[:])
                        nc.gpsimd.collective_compute(
                            "AllGather", Bypass,
                            ins=[spkb.opt()], outs=[gout.opt()],
                            replica_groups=[list(range(N_CORES))])
                        gt = []
                        for i in range(4):
                            gq = lgt.tile([HPC, 2 * B], b16, tag=f"gt{i}")
                            eng = nc.sync if i % 2 == 0 else nc.scalar
                            eng.dma_start(
                                gq[:].rearrange("p (k b) -> p k b", k=2),
                                gout[2 * i * HPC:2 * (i + 1) * HPC, :]
                                .rearrange("(k p) b -> p k b", k=2))
                            gt.append(gq)

                        # ---- z' for t+1 ----
                        nc.vector.scalar_tensor_tensor(
                            z_sb[:], in0=spk[:], scalar=nvoa_sb[:], in1=u[:],
                            op0=Mult, op1=Add)
                        nc.vector.tensor_tensor(z_sb[:], z_sb[:],
                                                pre[t + 1][1][:], op=Add)

                    # ---- dendritic drive vp (batch x feature) ----
                    vp = vpp.tile([B, FPC], f32, tag="vp")
                    nc.tensor.matmul(vp[:], lhsT=identb_sb[:], rhs=xdr[:],
                                     start=True, stop=(t == 0))
                    if t > 0:
                        for k in range(KT):
                            nc.tensor.matmul(
                                vp[:],
                                lhsT=gt_prev[k // 2][:, (k % 2) * B:
                                                     (k % 2 + 1) * B],
                                rhs=whT_sb[:, k * FPC:(k + 1) * FPC],
                                start=False, stop=(k == KT - 1))

                    # ---- e update + branch sums for next soma drive ----
                    nc.vector.tensor_tensor(e_sb[:], e_sb[:], betab_sb[:],
                                            op=Mult)
                    nc.vector.tensor_tensor(e_sb[:], e_sb[:], vp[:], op=Add)
                    if t < T - 1:
                        nc.vector.tensor_tensor(ce_sb[:], e_sb[:], cb_sb[:],
                                                op=Mult)
                        nc.vector.tensor_tensor(t1_sb[:], ce_sb[:, :2 * HPC],
                                                ce_sb[:, 2 * HPC:], op=Add)
                        s_sb = lss.tile([B, HPC], b16, tag="s")
                        nc.vector.tensor_tensor(s_sb[:], t1_sb[:, :HPC],
                                                t1_sb[:, HPC:], op=Add)
                        dp = dpp.tile([HPC, B], f32, tag="dp")
                        nc.tensor.matmul(dp[:], lhsT=s_sb[:],
                                         rhs=identb_sb[:],
                                         start=True, stop=False)

                    # ---- readout ----
                    rp = rpp.tile([HPC, OUT], f32, tag="rp")
                    for mo in range(2):
                        nc.tensor.matmul(
                            rp[:, mo * HPC:(mo + 1) * HPC],
                            lhsT=wroT_sb[:, mo * HPC:(mo + 1) * HPC],
                            rhs=spk[:], start=True, stop=True)
                    for mo in range(2):
                        nc.vector.scalar_tensor_tensor(
                            g_sb[:, mo * HPC:(mo + 1) * HPC],
                            in0=g_sb[:, mo * HPC:(mo + 1) * HPC],
                            scalar=alo_sb[:, mo:mo + 1],
                            in1=rp[:, mo * HPC:(mo + 1) * HPC],
                            op0=Mult, op1=Add)
                    if t >= WARMUP:
                        nc.vector.tensor_add(acc_sb[:], acc_sb[:], g_sb[:])

                    # ---- HAM-warmth fillers in the all-gather window ----
                    if 0 < t < T - 1:
                        for j in range(NFILL):
                            nc.tensor.matmul(
                                junk[:],
                                lhsT=wmemT_sb[:, (j % KT) * HPC:
                                              (j % KT + 1) * HPC],
                                rhs=whT_sb[:, (j % 8) * 256:
                                           (j % 8) * 256 + 256],
                                start=True, stop=True,
                                skip_group_check=True)

                    # ---- prefetch t+2 inputs (behind the gt loads) ----
                    if t + 2 < T:
                        pre.append(fetch_inputs(t + 2))

                    if t < T - 1:
                        gt_prev = gt

                junk_sb = spool.tile([HPC, 256], f32)
                nc.vector.tensor_copy(junk_sb[:], junk[:])
                nc.sync.dma_start(junk_out[:], junk_sb[:])

            nc.sync.dma_start(acc_out[:], acc_sb[:])

    nc.finalize()
    return nc


def _prep_inputs(x, W_dense, b_dense, mask, tau_n, tau_m, W_ro, b_ro, tau_m_ro):
    x = np.asarray(x, np.float32)
    eff_W = np.asarray(W_dense, np.float32) * np.asarray(mask, np.float32)
    b_dense = np.asarray(b_dense, np.float32)
    beta_f = _sigmoid(tau_n).reshape(-1)         # (4096,)
    alpha = _sigmoid(tau_m)                      # (1024,)
    alpha_o = _sigmoid(tau_m_ro)                 # (256,)
    W_ro = np.asarray(W_ro, np.float32)
    b_ro = np.asarray(b_ro, np.float32)

    Wx = eff_W[:, :IN_DIM]
    Wh = eff_W[:, IN_DIM:]
    xT = np.ascontiguousarray(
        x.transpose(2, 1, 0).reshape(IN_DIM, T * B)).astype(bf16)

    in_maps = []
    for c in range(N_CORES):
        fs = slice(c * FPC, (c + 1) * FPC)
        hs = slice(c * HPC, (c + 1) * HPC)
        al_h = alpha[hs]
        omal_h = 1.0 - al_h
        # branch-major local feature permutation: new j*HPC+h <- old h*BRANCH+j
        hl = np.arange(HPC)
        jj = np.arange(BRANCH)
        perm = (hl[None, :] * BRANCH + jj[:, None]).reshape(-1)  # (512,)
        ombeta = 1.0 - beta_f[fs]
        whp = (Wh[fs, :] * ombeta[:, None])[perm, :]     # (512, HID) bm order
        wxp = (Wx[fs, :] * ombeta[:, None])[perm, :]
        beta_p = beta_f[fs][perm]
        b_p = b_dense[fs][perm]
        wmem = (Wh[fs, :] * ombeta[:, None]).reshape(HPC, BRANCH, HID).sum(1) \
            * omal_h[:, None]
        wxmal = (Wx[fs, :] * ombeta[:, None]).reshape(HPC, BRANCH, IN_DIM).sum(1) \
            * (omal_h / al_h)[:, None]
        cvec = beta_p * np.tile(omal_h, BRANCH)          # (512,) (1-a)*beta
        wroT = np.ascontiguousarray((W_ro[:, hs] * (1.0 - alpha_o)[:, None]).T)
        c_h = b_dense[fs].reshape(HPC, BRANCH).sum(1)
        in_maps.append({
            "whT_in": np.ascontiguousarray(whp.T).astype(bf16),
            "wxT_in": np.ascontiguousarray(wxp.T).astype(bf16),
            "wmemT_in": np.ascontiguousarray(wmem.T).astype(bf16),
            "wxmemT_in": np.ascontiguousarray(wxmal.T).astype(bf16),
            "wroT_in": wroT.astype(bf16),
            "xT_in": xT,
            "einitT_in": np.ascontiguousarray(
                np.repeat(-b_p[None, :], B, 0)).astype(np.float32),
            "zinit_in": np.ascontiguousarray(
                np.repeat(-c_h[:, None], B, 1)).astype(np.float32),
            "alpha_in": al_h.reshape(HPC, 1).copy(),
            "nvoa_in": (-VTH / al_h).reshape(HPC, 1).astype(np.float32),
            "thr_in": (VTH - c_h).reshape(HPC, 1).astype(np.float32),
            "betab_in": np.ascontiguousarray(
                np.repeat(beta_p[None, :], B, 0)).astype(np.float32),
            "cb_in": np.ascontiguousarray(
                np.repeat(cvec[None, :], B, 0)).astype(np.float32),
            "alo_in": np.ascontiguousarray(
                alpha_o.reshape(2, HPC).T).copy(),
            "identb_in": np.eye(B, dtype=np.float32).astype(bf16),
        })

    tt = np.arange(WARMUP, T)
    bias_term = (b_ro.astype(np.float64)
                 * (1.0 - (np.asarray(alpha_o, np.float64)[None, :]
                           ** (tt[:, None] + 1)).mean(0))).astype(np.float32)
    return in_maps, bias_term


def run_kernel(trace=False, **inputs):
    from concourse import bass_utils

    in_maps, bias_term = _prep_inputs(**inputs)
    if "prog" not in _PROG_CACHE:
        _PROG_CACHE["prog"] = build_program()
    nc = _PROG_CACHE["prog"]
    res = bass_utils.run_bass_kernel_spmd(
        nc, in_maps, core_ids=list(range(N_CORES)), trace=trace)

    total = np.zeros((HPC, OUT), np.float32)
    for c in range(N_CORES):
        total += res.results[c]["acc_out"]
    part = total.reshape(HPC, 2, B).transpose(2, 1, 0).reshape(B, OUT)
    out = part / (T - WARMUP) + bias_term[None, :]
    return out.astype(np.float32), res


def kernel(**inputs):
    out, _ = run_kernel(trace=False, **inputs)
    return out


# revision 13
# speedup vs baseline: 1.1185x; 1.1185x over previous
"""Trainium2 Bass kernel for nn_DHSRNN (dendritic hierarchical spiking RNN).

Strategy: 8-way tensor-parallel over the HID*BRANCH=4096 dendritic feature dim
(512 feats / 128 hidden neurons per core), full batch (128) kept on every core.
Recurrent spikes are exchanged once per timestep with an AllGather of each
core's (128 hid x 128 batch) bf16 spike tile.

v2 layout (vs the earlier weight-stationary version): the gathered spike tile
gt is used as the PE *stationary* operand for the dendritic-drive matmul, with
the dense weights streaming as the moving operand (8 matmuls of free=512
instead of 32 weight-loads of free=128) -- the PE was LDWEIGHTS-bound.  The
dendritic state e lives transposed as (batch x feature); branch sums for the
soma drive are computed on the vector engine and injected into the soma PSUM
with a single transpose-matmul.  Only the 8 wmem matmuls + threshold sit on
the serial path between gather t-1 and gather t.

Math restructuring (exact, same as the validated baseline):
 - biases eliminated via state shifts e = d - b, f = mem - c; spike condition
   is a per-partition threshold f > VTH - c; readout bias re-added on host.
 - soft reset folded: z = f - (VTH/alpha)*spk, f_t = alpha*z_{t-1} + drive.
 - (1-beta) folded into dense weights; the soma drive is
   alpha*z + s(e_{t-1}) + wmem@spk_{t-1} + xm_t, where s is the
   (1-alpha)*beta branch sum of e and xm (x part) is precomputed; xm/alpha is
   folded into z' = z + xm_{t+1}/alpha so no PE identity-matmul is needed.
 - input drive (x @ Wx') for all 200 steps precomputed on-device in phase B;
   streamed back per-step via a transposing DMA into (batch x feature) layout.

Feature order within a core is branch-major (j*128 + h) so branch sums are
contiguous 128-column block adds on the vector engine.
"""
import sys

sys.path.insert(0, "/opt/trn_rl_repo")

import numpy as np
import ml_dtypes

IN_DIM, HID, OUT, BRANCH = 512, 1024, 256, 4
B, T = 128, 200
VTH, WARMUP = 1.0, 10
N_CORES = 8
FPC = HID * BRANCH // N_CORES   # 512 feats per core
HPC = HID // N_CORES            # 128 hid per core
NPIECE = T * B // 512           # 50 phase-B column pieces

bf16 = ml_dtypes.bfloat16

_PROG_CACHE = {}


def _sigmoid(x):
    return (1.0 / (1.0 + np.exp(-np.asarray(x, np.float64)))).astype(np.float32)


def build_program():
    from concourse import bacc, tile, mybir

    nc = bacc.Bacc("TRN2", target_bir_lowering=False, debug=False,
                   num_devices=N_CORES)
    f32 = mybir.dt.float32
    b16 = mybir.dt.bfloat16

    # ---- I/O ----
    whT_in = nc.dram_tensor("whT_in", [HID, FPC], b16, kind="ExternalInput").ap()
    wxT_in = nc.dram_tensor("wxT_in", [IN_DIM, FPC], b16, kind="ExternalInput").ap()
    wmemT_in = nc.dram_tensor("wmemT_in", [HID, HPC], b16, kind="ExternalInput").ap()
    wxmemT_in = nc.dram_tensor("wxmemT_in", [IN_DIM, HPC], b16, kind="ExternalInput").ap()
    wroT_in = nc.dram_tensor("wroT_in", [HPC, OUT], b16, kind="ExternalInput").ap()
    xT_in = nc.dram_tensor("xT_in", [IN_DIM, T * B], b16, kind="ExternalInput").ap()
    einitT_in = nc.dram_tensor("einitT_in", [B, FPC], f32, kind="ExternalInput").ap()
    zinit_in = nc.dram_tensor("zinit_in", [HPC, B], f32, kind="ExternalInput").ap()
    alpha_in = nc.dram_tensor("alpha_in", [HPC, 1], f32, kind="ExternalInput").ap()
    nvoa_in = nc.dram_tensor("nvoa_in", [HPC, 1], f32, kind="ExternalInput").ap()
    thr_in = nc.dram_tensor("thr_in", [HPC, 1], f32, kind="ExternalInput").ap()
    betab_in = nc.dram_tensor("betab_in", [B, FPC], f32, kind="ExternalInput").ap()
    cb_in = nc.dram_tensor("cb_in", [B, FPC], f32, kind="ExternalInput").ap()
    alo_in = nc.dram_tensor("alo_in", [HPC, 2], f32, kind="ExternalInput").ap()
    identb_in = nc.dram_tensor("identb_in", [B, B], b16, kind="ExternalInput").ap()

    acc_out = nc.dram_tensor("acc_out", [HPC, OUT], f32, kind="ExternalOutput").ap()
    junk_out = nc.dram_tensor("junk_out", [HPC, 256], f32, kind="ExternalOutput").ap()

    KT = HID // HPC       # 8 hid k-chunks
    KX = IN_DIM // HPC    # 4 input k-chunks
    MT = FPC // HPC       # 4 feat m-chunks
    Add = mybir.AluOpType.add
    Mult = mybir.AluOpType.mult
    IsGt = mybir.AluOpType.is_gt
    Bypass = mybir.AluOpType.bypass

    with tile.TileContext(nc) as tc:
        with tc.tile_pool(name="consts", bufs=1) as cpool, \
             tc.tile_pool(name="state", bufs=1) as spool, \
             tc.tile_pool(name="dramw", bufs=1, space="DRAM") as dpool:

            # ---- resident constants in SBUF ----
            whT_sb = cpool.tile([HPC, KT * FPC], b16)      # (128, 8*512)
            for k in range(KT):
                nc.sync.dma_start(whT_sb[:, k * FPC:(k + 1) * FPC],
                                  whT_in[k * HPC:(k + 1) * HPC, :])
            wmemT_sb = cpool.tile([HPC, KT * HPC], b16)    # (128, 8*128)
            for k in range(KT):
                nc.sync.dma_start(wmemT_sb[:, k * HPC:(k + 1) * HPC],
                                  wmemT_in[k * HPC:(k + 1) * HPC, :])
            wxT_sb = cpool.tile([HPC, KX * FPC], b16)      # (128, 4*512)
            for k in range(KX):
                nc.sync.dma_start(wxT_sb[:, k * FPC:(k + 1) * FPC],
                                  wxT_in[k * HPC:(k + 1) * HPC, :])
            wxmemT_sb = cpool.tile([HPC, KX * HPC], b16)   # (128, 4*128)
            for k in range(KX):
                nc.sync.dma_start(wxmemT_sb[:, k * HPC:(k + 1) * HPC],
                                  wxmemT_in[k * HPC:(k + 1) * HPC, :])
            wroT_sb = cpool.tile([HPC, OUT], b16)
            nc.sync.dma_start(wroT_sb[:], wroT_in[:])
            identb_sb = cpool.tile([B, B], b16)
            nc.sync.dma_start(identb_sb[:], identb_in[:])
            alpha_sb = cpool.tile([HPC, 1], f32)
            nc.sync.dma_start(alpha_sb[:], alpha_in[:])
            nvoa_sb = cpool.tile([HPC, 1], f32)
            nc.sync.dma_start(nvoa_sb[:], nvoa_in[:])
            thr_sb = cpool.tile([HPC, 1], f32)
            nc.sync.dma_start(thr_sb[:], thr_in[:])
            betab_sb = cpool.tile([B, FPC], f32)
            nc.scalar.dma_start(betab_sb[:], betab_in[:])
            cb_sb = cpool.tile([B, FPC], f32)
            nc.scalar.dma_start(cb_sb[:], cb_in[:])
            alo_sb = cpool.tile([HPC, 2], f32)
            nc.sync.dma_start(alo_sb[:], alo_in[:])

            # ---- persistent state ----
            e_sb = spool.tile([B, FPC], f32)               # (128 batch, 512 feat)
            nc.scalar.dma_start(e_sb[:], einitT_in[:])
            z_sb = spool.tile([HPC, B], f32)               # z' = z + xm/alpha
            nc.sync.dma_start(z_sb[:], zinit_in[:])
            ce_sb = spool.tile([B, FPC], f32)
            t1_sb = spool.tile([B, 2 * HPC], f32)
            g_sb = spool.tile([HPC, OUT], f32)
            nc.vector.memset(g_sb[:], 0.0)
            acc_sb = spool.tile([HPC, OUT], f32)
            nc.vector.memset(acc_sb[:], 0.0)

            # ---- internal DRAM ----
            xdrive_dram = dpool.tile([T, B, FPC], b16)
            xmal_dram = dpool.tile([T, HPC, B], b16)

            # ================= Phase B: x-drive precompute =================
            # xdrive comes out already (batch x feature): x-slice stationary,
            # weights moving.
            with tc.tile_pool(name="pbx", bufs=3) as pbx, \
                 tc.tile_pool(name="pbo", bufs=4) as pbo, \
                 tc.tile_pool(name="pbp", bufs=2, space="PSUM") as pbp:
                for p in range(NPIECE):
                    cs = p * 512
                    xtile = pbx.tile([HPC, KX * 512], b16)
                    nc.sync.dma_start(
                        xtile[:].rearrange("p (k c) -> p k c", k=KX),
                        xT_in[:, cs:cs + 512].rearrange("(k p) c -> p k c", k=KX))
                    for tl in range(4):
                        xdp = pbp.tile([B, FPC], f32, tag="xdp")
                        for k in range(KX):
                            nc.tensor.matmul(
                                xdp[:],
                                lhsT=xtile[:, k * 512 + tl * B:
                                           k * 512 + (tl + 1) * B],
                                rhs=wxT_sb[:, k * FPC:(k + 1) * FPC],
                                start=(k == 0), stop=(k == KX - 1))
                        xdo = pbo.tile([B, FPC], b16, tag="xdo")
                        nc.vector.tensor_copy(xdo[:], xdp[:])
                        nc.scalar.dma_start(xdrive_dram[4 * p + tl], xdo[:])
                    xmp = pbp.tile([HPC, 512], f32, tag="xmp")
                    for k in range(KX):
                        nc.tensor.matmul(
                            xmp[:],
                            lhsT=wxmemT_sb[:, k * HPC:(k + 1) * HPC],
                            rhs=xtile[:, k * 512:(k + 1) * 512],
                            start=(k == 0), stop=(k == KX - 1))
                    xmo = pbo.tile([HPC, 512], b16, tag="xmo")
                    nc.scalar.copy(xmo[:], xmp[:])
                    nc.gpsimd.dma_start(
                        xmal_dram[4 * p:4 * p + 4].rearrange("tl p b -> p tl b"),
                        xmo[:].rearrange("p (tl b) -> p tl b", tl=4))

            # ================= Phase C: recurrent loop =================
            NFILL = 10
            with tc.tile_pool(name="lio", bufs=4) as lio, \
                 tc.tile_pool(name="lgt", bufs=2) as lgt, \
                 tc.tile_pool(name="lfs", bufs=2) as lfs, \
                 tc.tile_pool(name="lss", bufs=2) as lss, \
                 tc.tile_pool(name="ldr", bufs=2, space="DRAM") as ldr, \
                 tc.tile_pool(name="vp", bufs=2, space="PSUM") as vpp, \
                 tc.tile_pool(name="dp", bufs=2, space="PSUM") as dpp, \
                 tc.tile_pool(name="jp", bufs=1, space="PSUM") as jpp, \
                 tc.tile_pool(name="rp", bufs=2, space="PSUM") as rpp:

                junk = jpp.tile([HPC, 256], f32, tag="junk")

                def fetch_inputs(t):
                    xdr = lio.tile([B, FPC], b16, tag="xdr")
                    nc.sync.dma_start(xdr[:], xdrive_dram[t])
                    xmal = lio.tile([HPC, B], b16, tag="xmal")
                    nc.scalar.dma_start(xmal[:], xmal_dram[t])
                    return xdr, xmal

                pre = [fetch_inputs(t) for t in range(2)]

                # initial z' = zinit + xmal_0
                nc.vector.tensor_tensor(z_sb[:], z_sb[:], pre[0][1][:], op=Add)

                # initial s from e_init, into dp for t=0 (no wmem at t=0)
                nc.vector.tensor_tensor(ce_sb[:], e_sb[:], cb_sb[:], op=Mult)
                nc.vector.tensor_tensor(t1_sb[:], ce_sb[:, :2 * HPC],
                                        ce_sb[:, 2 * HPC:], op=Add)
                s_sb = lss.tile([B, HPC], b16, tag="s")
                nc.vector.tensor_tensor(s_sb[:], t1_sb[:, :HPC],
                                        t1_sb[:, HPC:], op=Add)
                dp = dpp.tile([HPC, B], f32, tag="dp")
                nc.tensor.matmul(dp[:], lhsT=s_sb[:], rhs=identb_sb[:],
                                 start=True, stop=True)

                gt_prev = None   # list of 4 tiles, each [HPC, 2*B]
                for t in range(T):
                    xdr, xmal = pre[t]

                    # ---- soma drive: wmem @ gathered spikes (t-1) ----
                    if t > 0:
                        for k in range(KT):
                            nc.tensor.matmul(
                                dp[:],
                                lhsT=wmemT_sb[:, k * HPC:(k + 1) * HPC],
                                rhs=gt_prev[k // 2][:, (k % 2) * B:
                                                    (k % 2 + 1) * B],
                                start=False, stop=(k == KT - 1))

                    # ---- f, spike ----
                    u = lfs.tile([HPC, B], f32, tag="u")
                    nc.vector.scalar_tensor_tensor(
                        u[:], in0=z_sb[:], scalar=alpha_sb[:], in1=dp[:],
                        op0=Mult, op1=Add)
                    spk = lfs.tile([HPC, B], b16, tag="spk")
                    nc.vector.tensor_scalar(spk[:], u[:], thr_sb[:], None,
                                            op0=IsGt)

                    # ---- bounce + all-gather + chunked reload ----
                    if t < T - 1:
                        spkb = ldr.tile([HPC, B], b16, tag="spkb")
                        gout = ldr.tile([HID, B], b16, tag="gout",
                                        addr_space="Shared")
                        nc.scalar.dma_start(spkb[:], spk[:])
                        nc.gpsimd.collective_compute(
                            "AllGather", Bypass,
                            ins=[spkb.opt()], outs=[gout.opt()],
                            replica_groups=[list(range(N_CORES))])
                        gt = []
                        for i in range(4):
                            gq = lgt.tile([HPC, 2 * B], b16, tag=f"gt{i}")
                            eng = nc.sync if i % 2 == 0 else nc.scalar
                            eng.dma_start(
                                gq[:].rearrange("p (k b) -> p k b", k=2),
                                gout[2 * i * HPC:2 * (i + 1) * HPC, :]
                                .rearrange("(k p) b -> p k b", k=2))
                            gt.append(gq)

                        # ---- z' for t+1 ----
                        nc.vector.scalar_tensor_tensor(
                            z_sb[:], in0=spk[:], scalar=nvoa_sb[:], in1=u[:],
                            op0=Mult, op1=Add)
                        nc.vector.tensor_tensor(z_sb[:], z_sb[:],
                                                pre[t + 1][1][:], op=Add)

                    # ---- dendritic drive vp (batch x feature) ----
                    vp = vpp.tile([B, FPC], f32, tag="vp")
                    nc.tensor.matmul(vp[:], lhsT=identb_sb[:], rhs=xdr[:],
                                     start=True, stop=(t == 0))
                    if t > 0:
                        for k in range(KT):
                            nc.tensor.matmul(
                                vp[:],
                                lhsT=gt_prev[k // 2][:, (k % 2) * B:
                                                     (k % 2 + 1) * B],
                                rhs=whT_sb[:, k * FPC:(k + 1) * FPC],
                                start=False, stop=(k == KT - 1))

                    # ---- e update + branch sums for next soma drive ----
                    nc.vector.tensor_tensor(e_sb[:], e_sb[:], betab_sb[:],
                                            op=Mult)
                    nc.vector.tensor_tensor(e_sb[:], e_sb[:], vp[:], op=Add)
                    if t < T - 1:
                        nc.vector.tensor_tensor(ce_sb[:], e_sb[:], cb_sb[:],
                                                op=Mult)
                        nc.vector.tensor_tensor(t1_sb[:], ce_sb[:, :2 * HPC],
                                                ce_sb[:, 2 * HPC:], op=Add)
                        s_sb = lss.tile([B, HPC], b16, tag="s")
                        nc.vector.tensor_tensor(s_sb[:], t1_sb[:, :HPC],
                                                t1_sb[:, HPC:], op=Add)
                        dp = dpp.tile([HPC, B], f32, tag="dp")
                        nc.tensor.matmul(dp[:], lhsT=s_sb[:],
                                         rhs=identb_sb[:],
                                         start=True, stop=False)

                    # ---- readout ----
                    rp = rpp.tile([HPC, OUT], f32, tag="rp")
                    for mo in range(2):
                        nc.tensor.matmul(
                            rp[:, mo * HPC:(mo + 1) * HPC],
                            lhsT=wroT_sb[:, mo * HPC:(mo + 1) * HPC],
                            rhs=spk[:], start=True, stop=True)
                    for mo in range(2):
                        nc.vector.scalar_tensor_tensor(
                            g_sb[:, mo * HPC:(mo + 1) * HPC],
                            in0=g_sb[:, mo * HPC:(mo + 1) * HPC],
                            scalar=alo_sb[:, mo:mo + 1],
                            in1=rp[:, mo * HPC:(mo + 1) * HPC],
                            op0=Mult, op1=Add)
                    if t >= WARMUP:
                        nc.vector.tensor_add(acc_sb[:], acc_sb[:], g_sb[:])

                    # ---- HAM-warmth fillers in the all-gather window ----
                    if 0 < t < T - 1:
                        for j in range(NFILL):
                            nc.tensor.matmul(
                                junk[:],
                                lhsT=wmemT_sb[:, (j % KT) * HPC:
                                              (j % KT + 1) * HPC],
                                rhs=whT_sb[:, (j % 8) * 256:
                                           (j % 8) * 256 + 256],
                                start=True, stop=True,
                                skip_group_check=True)

                    # ---- prefetch t+2 inputs (behind the gt loads) ----
                    if t + 2 < T:
                        pre.append(fetch_inputs(t + 2))

                    if t < T - 1:
                        gt_prev = gt

                junk_sb = spool.tile([HPC, 256], f32)
                nc.vector.tensor_copy(junk_sb[:], junk[:])
                nc.sync.dma_start(junk_out[:], junk_sb[:])

            nc.sync.dma_start(acc_out[:], acc_sb[:])

    nc.finalize()
    return nc


def _prep_inputs(x, W_dense, b_dense, mask, tau_n, tau_m, W_ro, b_ro, tau_m_ro):
    x = np.asarray(x, np.float32)
    eff_W = np.asarray(W_dense, np.float32) * np.asarray(mask, np.float32)
    b_dense = np.asarray(b_dense, np.float32)
    beta_f = _sigmoid(tau_n).reshape(-1)         # (4096,)
    alpha = _sigmoid(tau_m)                      # (1024,)
    alpha_o = _sigmoid(tau_m_ro)                 # (256,)
    W_ro = np.asarray(W_ro, np.float32)
    b_ro = np.asarray(b_ro, np.float32)

    Wx = eff_W[:, :IN_DIM]
    Wh = eff_W[:, IN_DIM:]
    xT = np.ascontiguousarray(
        x.transpose(2, 1, 0).reshape(IN_DIM, T * B)).astype(bf16)

    in_maps = []
    for c in range(N_CORES):
        fs = slice(c * FPC, (c + 1) * FPC)
        hs = slice(c * HPC, (c + 1) * HPC)
        al_h = alpha[hs]
        omal_h = 1.0 - al_h
        # branch-major local feature permutation: new j*HPC+h <- old h*BRANCH+j
        hl = np.arange(HPC)
        jj = np.arange(BRANCH)
        perm = (hl[None, :] * BRANCH + jj[:, None]).reshape(-1)  # (512,)
        ombeta = 1.0 - beta_f[fs]
        whp = (Wh[fs, :] * ombeta[:, None])[perm, :]     # (512, HID) bm order
        wxp = (Wx[fs, :] * ombeta[:, None])[perm, :]
        beta_p = beta_f[fs][perm]
        b_p = b_dense[fs][perm]
        wmem = (Wh[fs, :] * ombeta[:, None]).reshape(HPC, BRANCH, HID).sum(1) \
            * omal_h[:, None]
        wxmal = (Wx[fs, :] * ombeta[:, None]).reshape(HPC, BRANCH, IN_DIM).sum(1) \
            * (omal_h / al_h)[:, None]
        cvec = beta_p * np.tile(omal_h, BRANCH)          # (512,) (1-a)*beta
        wroT = np.ascontiguousarray((W_ro[:, hs] * (1.0 - alpha_o)[:, None]).T)
        c_h = b_dense[fs].reshape(HPC, BRANCH).sum(1)
        in_maps.append({
            "whT_in": np.ascontiguousarray(whp.T).astype(bf16),
            "wxT_in": np.ascontiguousarray(wxp.T).astype(bf16),
            "wmemT_in": np.ascontiguousarray(wmem.T).astype(bf16),
            "wxmemT_in": np.ascontiguousarray(wxmal.T).astype(bf16),
            "wroT_in": wroT.astype(bf16),
            "xT_in": xT,
            "einitT_in": np.ascontiguousarray(
                np.repeat(-b_p[None, :], B, 0)).astype(np.float32),
            "zinit_in": np.ascontiguousarray(
                np.repeat(-c_h[:, None], B, 1)).astype(np.float32),
            "alpha_in": al_h.reshape(HPC, 1).copy(),
            "nvoa_in": (-VTH / al_h).reshape(HPC, 1).astype(np.float32),
            "thr_in": (VTH - c_h).reshape(HPC, 1).astype(np.float32),
            "betab_in": np.ascontiguousarray(
                np.repeat(beta_p[None, :], B, 0)).astype(np.float32),
            "cb_in": np.ascontiguousarray(
                np.repeat(cvec[None, :], B, 0)).astype(np.float32),
            "alo_in": np.ascontiguousarray(
                alpha_o.reshape(2, HPC).T).copy(),
            "identb_in": np.eye(B, dtype=np.float32).astype(bf16),
        })

    tt = np.arange(WARMUP, T)
    bias_term = (b_ro.astype(np.float64)
                 * (1.0 - (np.asarray(alpha_o, np.float64)[None, :]
                           ** (tt[:, None] + 1)).mean(0))).astype(np.float32)
    return in_maps, bias_term


def run_kernel(trace=False, **inputs):
    from concourse import bass_utils

    in_maps, bias_term = _prep_inputs(**inputs)
    if "prog" not in _PROG_CACHE:
        _PROG_CACHE["prog"] = build_program()
    nc = _PROG_CACHE["prog"]
    res = bass_utils.run_bass_kernel_spmd(
        nc, in_maps, core_ids=list(range(N_CORES)), trace=trace)

    total = np.zeros((HPC, OUT), np.float32)
    for c in range(N_CORES):
        total += res.results[c]["acc_out"]
    part = total.reshape(HPC, 2, B).transpose(2, 1, 0).reshape(B, OUT)
    out = part / (T - WARMUP) + bias_term[None, :]
    return out.astype(np.float32), res


def kernel(**inputs):
    out, _ = run_kernel(trace=False, **inputs)
    return out
